# revision 25
# baseline (speedup 1.0000x reference)
"""AdditiveAttention (Bahdanau) on 8 TRN2 NeuronCores.

score[b,q,k] = sum_h wv[h] * tanh(q_proj[b,q,h] + k_proj[b,k,h])
out = softmax_k(masked score) @ value

Sharding: data-parallel over queries, balanced across batches — every core
processes rows 32c..32c+32 of EVERY batch (32 rows x 4 batches = 128 rows),
so all 8 cores run the identical instruction stream on different data and
no collectives are needed.  Keys beyond valid_len[b] are skipped entirely
at graph-build time (exact: the reference's -1e6 mask underflows exp to
0.0 in f32, so invalid keys contribute nothing).

Per core, per query row r of batch b (layout: h on partitions, keys free):
  ACT: feat_ht = tanh(k_projT[ht][:, :v_b] + bias q_projT[ht][:, r])  (bf16)
  PE : score[r, :v_b] += wv_ht.T @ feat_ht   (M=1 matmuls into PSUM row r)
then per batch group (32 rows at partition offset 32g): reduce_max,
exp(bias=-max, accum_out=sumexp), reciprocal, PE transpose of attn,
attn @ value, scale rows by 1/sumexp.
"""

import numpy as np
import ml_dtypes

try:  # make trace-enabled environments degrade gracefully instead of crashing
    import antenv.axon_hooks  # noqa: F401
except ImportError:
    import sys as _sys
    import types as _types

    _m = _types.ModuleType("antenv.axon_hooks")
    _m.get_axon_ntff_profile_hook = lambda: None
    _m.set_axon_ntff_profile_hook = lambda h: None
    _sys.modules["antenv.axon_hooks"] = _m

import concourse.bass as bass
import concourse.tile as tile
from concourse import mybir
from concourse.vector_clock import ScopedClock
from concourse.bass_utils import run_bass_kernel_spmd
from concourse.masks import make_identity

BF16 = ml_dtypes.bfloat16
NCORES = 8
RPB = 32  # rows per batch per core


class _TC(tile.TileContext):
    """Tail drain can exceed walrus's per-instruction sync-wait slots;
    move the waits onto standalone SP wait ops."""

    def _drain_and_barrier(self, tick_clock, wait_clock):
        nc = self.nc
        drain_inst = nc.sync.drain()
        wait_clock.add_sem_waits(
            drain_inst.ins, ScopedClock({None: tick_clock.global_clock})
        )
        waits = list(drain_inst.ins.sync_info.on_wait)
        if len(waits) > 1:
            drain_inst.ins.sync_info.on_wait = []
            assert self.sems is not None
            by_name = {h.name: h for h in self.sems.allocated().values()}
            for w in waits:
                assert w.wait_mode == "sem-ge-imm", w
                nc.sync.wait_ge(by_name[w.ant_name], w.wait_value)
        nc.all_engine_barrier()
        assert self.sems is not None
        popped = nc._tile_sem_poison_stack.pop()
        assert popped is self._sem_poison
        nc.clear_and_free_semaphores(list(self.sems.allocated().values()))
        nc.all_engine_barrier()


def _ceil(a, m):
    return (a + m - 1) // m * m


_ENGINE_TYPES = {
    mybir.EngineType.PE,
    mybir.EngineType.Activation,
    mybir.EngineType.DVE,
    mybir.EngineType.Pool,
    mybir.EngineType.SP,
}


def _split_excess_waits(nc, maxw=2):
    """walrus's per-instruction sync-wait slots are tiny; hoist excess waits
    onto same-engine NOP carriers inserted just before the instruction."""
    for f in nc.m.functions:
        for bb in f.blocks:
            insts = list(bb.instructions)
            out, changed = [], False
            for inst in insts:
                si = inst.sync_info
                nw = len(si.on_wait) if si is not None and si.on_wait else 0
                if nw > maxw and inst.engine in _ENGINE_TYPES:
                    waits = list(si.on_wait)
                    keep, excess = waits[:1], waits[1:]
                    for w in excess:
                        bi = nc.engines[inst.engine].nop()
                        carrier = bi.ins
                        tail = nc.cur_bb.bb
                        tail.instructions = [
                            i for i in tail.instructions if i.name != carrier.name
                        ]
                        import bass_rust

                        carrier.sync_info = bass_rust.SyncInfo(
                            on_wait=[w], on_update=[]
                        )
                        out.append(carrier)
                        changed = True
                    inst.sync_info.on_wait = keep
                out.append(inst)
            if changed:
                bb.instructions = out


def _build(vlist, dq, dh, dv):
    """Build the SPMD graph for per-batch valid lengths vlist."""
    f32, bf16 = mybir.dt.float32, mybir.dt.bfloat16
    nb = len(vlist)
    kt = sum(vlist)  # total valid keys (unpadded, for keyT / k_projT)
    koff = np.cumsum([0] + list(vlist)).tolist()
    vpad = [_ceil(v, 128) for v in vlist]  # padded for value/attnT tiles
    toff = np.cumsum([0] + [v // 128 for v in vpad]).tolist()
    tt = toff[-1]  # total 128-key tiles
    vmax = max(vlist)
    vpmax = max(vpad)
    nct = dq // 128  # contraction tiles for projections
    nht = dh // 128  # h tiles
    nrows = nb * RPB

    nc = bass.Bass()
    keyT_e = nc.declare_dram_parameter("keyT", [128, nct, kt], bf16, isOutput=False)
    val_e = nc.declare_dram_parameter("val", [128, tt, dv], bf16, isOutput=False)
    qT_e = nc.declare_dram_parameter("qT", [128, nct, nrows], bf16, isOutput=False)
    wq_e = nc.declare_dram_parameter("wq", [128, nct, dh], bf16, isOutput=False)
    wk_e = nc.declare_dram_parameter("wk", [128, nct, dh], bf16, isOutput=False)
    wv_e = nc.declare_dram_parameter("wv", [128, nht], bf16, isOutput=False)
    out_e = nc.declare_dram_parameter("out", [nrows, dv], f32, isOutput=True)

    with _TC(nc) as tc:
        sg = tc.alloc_tile_pool(name="singles", bufs=1)
        feat = tc.alloc_tile_pool(name="feat", bufs=24)
        pp = tc.alloc_tile_pool(name="pproj", bufs=2, space="PSUM")

        keyT = sg.tile([128, nct, kt], bf16)
        val = sg.tile([128, tt, dv], bf16)
        qT = sg.tile([128, nct, nrows], bf16)
        wqs = sg.tile([128, nct, dh], bf16)
        wks = sg.tile([128, nct, dh], bf16)
        wvs = sg.tile([128, nht], bf16)
        ident = sg.tile([128, 128], bf16)
        kpT = sg.tile([128, nht, kt], bf16)
        qpT = sg.tile([128, nht, nrows], f32)
        attns = [
            sg.tile([RPB, vpad[g]], bf16, tag=f"attn{g}", name=f"attn{g}")
            for g in range(nb)
        ]
        attnT = sg.tile([128, tt, RPB], bf16)
        outs = sg.tile([nrows, dv], f32)
        mx = sg.tile([128, 1], f32)
        mxn = sg.tile([128, 1], f32)
        se = sg.tile([128, 1], f32)
        rinv = sg.tile([128, 1], f32)

        order = sorted(range(nb), key=lambda g: vlist[g])
        order = [order[0]] + order[1:][::-1]  # smallest first, smallest-ish last
        # critical-path DMAs issued from four different engines so their
        # ~0.6us DGE issue costs run in parallel, not serially on SP
        g0 = order[0]
        nc.sync.dma_start(
            out=keyT[:, :, koff[g0] : koff[g0 + 1]],
            in_=keyT_e[:, :, koff[g0] : koff[g0 + 1]],
        )
        nc.scalar.dma_start(out=wqs, in_=wq_e[:])
        nc.sync.dma_start(out=qT, in_=qT_e[:])
        nc.scalar.dma_start(out=wks, in_=wk_e[:])
        for g in order[1:]:
            nc.sync.dma_start(
                out=keyT[:, :, koff[g] : koff[g + 1]],
                in_=keyT_e[:, :, koff[g] : koff[g + 1]],
            )
        nc.sync.dma_start(out=wvs, in_=wv_e[:])
        for g in order:
            nc.sync.dma_start(
                out=val[:, toff[g] : toff[g + 1], :],
                in_=val_e[:, toff[g] : toff[g + 1], :],
            )
        make_identity(nc, ident)
        for g in range(nb):
            nc.gpsimd.memset(attns[g], 0.0)

        def q_proj(ht):
            p = pp.tile([128, 512], f32, tag="proj", name="qp")
            for ct in range(nct):
                nc.tensor.matmul(
                    p[:, 0:nrows],
                    lhsT=wqs[:, ct, ht * 128 : (ht + 1) * 128],
                    rhs=qT[:, ct, :],
                    start=(ct == 0),
                    stop=(ct == nct - 1),
                )
            nc.vector.tensor_copy(out=qpT[:, ht, :], in_=p[:, 0:nrows])

        def k_proj(g, hts=None):
            v = vlist[g]
            for ht in hts if hts is not None else range(nht):
                for c0 in range(0, v, 512):
                    cl = min(512, v - c0)
                    p = pp.tile([128, 512], f32, tag="proj", name="kp")
                    for ct in range(nct):
                        nc.tensor.matmul(
                            p[:, 0:cl],
                            lhsT=wks[:, ct, ht * 128 : (ht + 1) * 128],
                            rhs=keyT[:, ct, koff[g] + c0 : koff[g] + c0 + cl],
                            start=(ct == 0),
                            stop=(ct == nct - 1),
                        )
                    nc.vector.tensor_copy(
                        out=kpT[:, ht, koff[g] + c0 : koff[g] + c0 + cl],
                        in_=p[:, 0:cl],
                    )

        # shortest path to the first tanh: ht0 projections first
        q_proj(0)
        k_proj(order[0], hts=[0])
        q_proj(1)
        k_proj(order[0], hts=[1])

        prow = tc.alloc_tile_pool(name="prow", bufs=3, space="PSUM")
        pt = tc.alloc_tile_pool(name="ptr", bufs=1, space="PSUM")
        po = tc.alloc_tile_pool(name="pout", bufs=2, space="PSUM")
        rbp = tc.alloc_tile_pool(name="rbp", bufs=2)
        score = sg.tile([128, vmax], f32)

        def softmax_epilogue(g):
            v = vlist[g]
            lo, hi = g * RPB, (g + 1) * RPB
            nc.vector.reduce_max(
                out=mx[lo:hi], in_=score[lo:hi, 0:v], axis=mybir.AxisListType.X
            )
            nc.vector.tensor_scalar_mul(mxn[lo:hi], mx[lo:hi], -1.0)
            nc.scalar.activation(
                out=attns[g][:, 0:v],
                in_=score[lo:hi, 0:v],
                func=mybir.ActivationFunctionType.Exp,
                bias=mxn[lo:hi],
                accum_out=se[lo:hi],
            )
            nc.vector.reciprocal(out=rinv[lo:hi], in_=se[lo:hi])

            # attn^T via PE transpose, then attn @ value
            ntile = vpad[g] // 128
            for t in range(ntile):
                ptr = pt.tile([128, RPB], bf16, tag="tr")
                nc.tensor.transpose(
                    out=ptr,
                    in_=attns[g][:, t * 128 : (t + 1) * 128],
                    identity=ident[0:RPB, 0:RPB],
                )
                nc.vector.tensor_copy(out=attnT[:, toff[g] + t, :], in_=ptr)
            op = po.tile([RPB, dv], f32, tag="out")
            for t in range(ntile):
                nc.tensor.matmul(
                    op,
                    lhsT=attnT[:, toff[g] + t, :],
                    rhs=val[:, toff[g] + t, :],
                    start=(t == 0),
                    stop=(t == ntile - 1),
                )
            nc.vector.tensor_scalar(
                out=outs[lo:hi, :],
                in0=op,
                scalar1=rinv[lo:hi],
                scalar2=None,
                op0=mybir.AluOpType.mult,
            )
            nc.sync.dma_start(out=out_e[lo:hi, :], in_=outs[lo:hi, :])

        RB = 8  # rows per scatter DMA (amortizes SP DMA-issue overhead)
        pending = None
        for g in order:
            v = vlist[g]
            lo, hi = g * RPB, (g + 1) * RPB
            if g != order[0]:
                k_proj(g)
            # scores for this group's 32 rows: per-row M=1 matmuls land in a
            # base-0 PSUM row tile (PE can only write quadrant bases), DVE
            # stages 8 rows in an SBUF buffer, then one SBUF->SBUF DMA
            # scatters them onto partitions lo+r.. of the score block
            # (engines can only address quadrant base partitions; DMA is
            # unrestricted).
            rb = None
            for r in range(RPB):
                row = lo + r
                fts = []
                for ht in range(nht):
                    f = feat.tile([128, vmax], bf16, tag="feat")
                    nc.scalar.activation(
                        out=f[:, 0:v],
                        in_=kpT[:, ht, koff[g] : koff[g] + v],
                        func=mybir.ActivationFunctionType.Tanh,
                        bias=qpT[:, ht, row : row + 1],
                    )
                    fts.append(f)
                if rb is None:
                    rb = rbp.tile([1, RB, vmax], f32, tag="rb")
                for c0 in range(0, v, 512):
                    cl = min(512, v - c0)
                    rt = prow.tile([1, 512], f32, tag="row")
                    for ht in range(nht):
                        nc.tensor.matmul(
                            rt[0:1, 0:cl],
                            lhsT=wvs[:, ht : ht + 1],
                            rhs=fts[ht][:, c0 : c0 + cl],
                            start=(ht == 0),
                            stop=(ht == nht - 1),
                        )
                    nc.vector.tensor_copy(
                        out=rb[0:1, r % RB, c0 : c0 + cl], in_=rt[0:1, 0:cl]
                    )
                if r % RB == RB - 1:
                    nc.sync.dma_start(
                        out=score[row - RB + 1 : row + 1, 0:v],
                        in_=rb[0:1, :, 0:v],
                    )
                    rb = None
                # previous group's softmax goes into the instruction streams
                # a couple of rows in, so its dependency waits don't stall ACT
                if r == 3 and pending is not None:
                    softmax_epilogue(pending)
                    pending = None
            pending = g
        softmax_epilogue(pending)

        for pool in (rbp, po, pt, prow, pp, feat, sg):
            pool.release()

    _split_excess_waits(nc, maxw=1)
    return nc


_cache = {}


def kernel(query, key, value, valid_len, Wq, Wk, wv):
    query = np.asarray(query, dtype=np.float32)
    key = np.asarray(key, dtype=np.float32)
    value = np.asarray(value, dtype=np.float32)
    Wq = np.asarray(Wq, dtype=np.float32)
    Wk = np.asarray(Wk, dtype=np.float32)
    wv = np.asarray(wv, dtype=np.float32)
    vl = np.asarray(valid_len).astype(np.int64)

    b, lq, dq = query.shape
    _, lk, dk = key.shape
    dv = value.shape[2]
    dh = Wq.shape[1]
    vlist = [max(1, min(int(x), lk)) for x in vl]

    nct, nht = dq // 128, dh // 128
    kt = sum(vlist)
    koff = np.cumsum([0] + vlist).tolist()
    vpad = [_ceil(v, 128) for v in vlist]
    toff = np.cumsum([0] + [v // 128 for v in vpad]).tolist()
    tt = toff[-1]
    nrows = b * RPB

    # replicated inputs, pre-laid-out for SBUF ([partition, tile, free])
    keyT_h = np.zeros((128, nct, kt), dtype=BF16)
    val_h = np.zeros((128, tt, dv), dtype=BF16)
    for g in range(b):
        kTg = key[g, : vlist[g], :].T  # [dq, v]
        keyT_h[:, :, koff[g] : koff[g + 1]] = (
            kTg.reshape(nct, 128, vlist[g]).transpose(1, 0, 2).astype(BF16)
        )
        vg_p = np.zeros((vpad[g], dv), dtype=np.float32)
        vg_p[: vlist[g]] = value[g, : vlist[g], :]
        val_h[:, toff[g] : toff[g + 1], :] = (
            vg_p.reshape(-1, 128, dv).transpose(1, 0, 2).astype(BF16)
        )
    wq_h = Wq.reshape(nct, 128, dh).transpose(1, 0, 2).astype(BF16)
    wk_h = Wk.reshape(nct, 128, dh).transpose(1, 0, 2).astype(BF16)
    wv_h = np.ascontiguousarray(wv.reshape(nht, 128).T).astype(BF16)

    ckey = tuple(vlist) + (dq, dh, dv)
    if ckey not in _cache:
        _cache[ckey] = _build(vlist, dq, dh, dv)
    nc = _cache[ckey]

    in_maps = []
    for c in range(NCORES):
        qrows = np.concatenate(
            [query[g, RPB * c : RPB * (c + 1), :] for g in range(b)], axis=0
        )  # [nrows, dq]
        qT_h = qrows.T.reshape(nct, 128, nrows).transpose(1, 0, 2).astype(BF16)
        in_maps.append(
            {
                "keyT": keyT_h,
                "val": val_h,
                "qT": np.ascontiguousarray(qT_h),
                "wq": np.ascontiguousarray(wq_h),
                "wk": np.ascontiguousarray(wk_h),
                "wv": wv_h,
            }
        )

    res = None
    for attempt in range(3):
        try:
            res = run_bass_kernel_spmd(nc, in_maps, core_ids=list(range(NCORES)))
            break
        except Exception:
            if attempt == 2:
                raise
            import time as _time

            _time.sleep(5.0)

    out = np.empty((b, lq, dv), dtype=np.float32)
    for c in range(NCORES):
        r = res.results[c]["out"]
        for g in range(b):
            out[g, RPB * c : RPB * (c + 1), :] = r[g * RPB : (g + 1) * RPB, :]
    return out
